# revision 1
# baseline (speedup 1.0000x reference)
"""GAT (2-layer, 6-head) + GraphNorm + readout MLP on 8 Trainium2 cores.

Sharding: graph-level data parallelism. 48 fixed-size graphs (228 nodes,
edges never cross graphs) -> 6 graphs per core. Weights replicated.

v2 redesign vs the per-graph baseline:
  - All 6 graphs batched per stage; channel-major [c, (g, n)] primary layout.
  - Dense attention scores z[s,d] built with GpSimd partition-broadcast of
    the a2 rows + one wide DVE add per graph ([114, 2*6*228] bf16 tiles,
    both source-halves per instruction), lrelu on DVE, exp on Scalar
    (single activation table: exp/ln/relu/copy), multiplicity mask on GpSimd.
  - Attention logits a1/a2 computed straight from the layer input with
    host-folded was = W @ [as|ad].
  - Aggregation: dest-partition matmuls with a fused ones-column so the
    softmax denominator falls out of the same PSUM tile.
  - lin1 readout: weights host-reordered to (ck, n, p) chunks of k=128 so
    the GEMV consumes the channel-major layer-2 output directly; weights
    streamed in 4 double-buffered DMA chunks overlapping the layer phase.

kernel(**inputs) -> np.ndarray [48, 2] float32.
"""
import sys
sys.path.insert(0, '/opt/trn_rl_repo')

import numpy as np

import concourse.bass as bass
import concourse.bacc as bacc
import concourse.mybir as mybir
import concourse.tile as tile
from concourse import masks
from concourse import bass_utils

F32 = mybir.dt.float32
BF16 = mybir.dt.bfloat16
Alu = mybir.AluOpType
Act = mybir.ActivationFunctionType

H, C = 6, 64
HC = 384
NPG = 228          # nodes per graph
B = 48             # graphs
GPC = 6            # graphs per core
NCORES = 8
F_IN = 228
NH = 114           # node half-chunk
NCLS = 2
NG = GPC * NPG     # 1368 node-columns per core
NJ1 = 3 * NPG      # 684 lin1 k-chunks of 128
NLCH = 12          # lin1 weight stream chunks
JPC = NJ1 // NLCH  # 171 chunks per stream piece

_last_results = {"exec_time_ns": None}


def _ensure_axon_hooks():
    """Make BASS_TRACE-driven NTFF profiling under axon degrade gracefully."""
    try:
        import antenv.axon_hooks  # noqa: F401
        return
    except ImportError:
        pass
    import types
    try:
        import antenv
    except ImportError:
        return
    mod = types.ModuleType("antenv.axon_hooks")
    holder = {"hook": None}
    mod.set_axon_ntff_profile_hook = lambda h: holder.__setitem__("hook", h)
    mod.get_axon_ntff_profile_hook = lambda: holder["hook"]
    sys.modules["antenv.axon_hooks"] = mod
    antenv.axon_hooks = mod
    try:
        from trn_agent_boot.trn_boot import _ntff_profile_via_ctypes
        hook = _ntff_profile_via_ctypes('/opt/axon/libaxon_pjrt.so')
        if hook is not None:
            mod.set_axon_ntff_profile_hook(hook)
    except Exception:
        pass
    _orig_upload = bass_utils.upload_artifacts

    def _safe_upload(tmpdir):
        try:
            return _orig_upload(tmpdir)
        except Exception:
            return "local://" + str(tmpdir)

    bass_utils.upload_artifacts = _safe_upload


_ensure_axon_hooks()


def _build_program():
    nc = bacc.Bacc("TRN2", target_bir_lowering=False, debug=False)

    dt_in = {}

    def din(name, shape, dtype=F32):
        t = nc.dram_tensor(name, shape, dtype, kind="ExternalInput")
        dt_in[name] = t
        return t

    din("xb", [NH, 2 * NG], BF16)            # x chan-major [p, (fc, g, n)]
    din("mm", [NH, 2 * NG], BF16)            # multiplicity+I [p, (sc, g, d)]
    din("w1s", [NH, 2 * HC], BF16)           # W1 [p, (fc, 384)]
    din("w2s", [128, 3 * HC], BF16)          # W2 [p, (kc, 384)]
    din("was1", [NH, 2 * 12], BF16)          # W1@[as|ad] [p, (fc, 12)]
    din("was2", [128, 3 * 12], BF16)
    din("gncol", [128, 4], F32)              # graphnorm gamma, col ck
    din("gncol2", [128, 4], F32)
    din("lin1s", [128, NJ1 * C], BF16)       # lin1_w reordered (p, (ck, n, 64))
    din("head64", [C, 4], F32)               # cols: lin1_b, bn_scale, bn_shift
    din("lin2w", [C, NCLS], F32)
    din("lin2b", [NCLS, 1], F32)

    out_d = nc.dram_tensor("out", [NCLS, GPC], F32, kind="ExternalOutput")

    with tile.TileContext(nc) as tc:
        _emit(tc, dt_in, out_d)

    nc.finalize()
    return nc


def _emit(tc, din, out_d):
    nc = tc.nc

    cst = tc.alloc_tile_pool(name="cst", bufs=1)
    lw = tc.alloc_tile_pool(name="lw", bufs=2)
    hp = tc.alloc_tile_pool(name="hp", bufs=1)
    att = tc.alloc_tile_pool(name="att", bufs=1)
    scp = tc.alloc_tile_pool(name="scp", bufs=2)
    agw = tc.alloc_tile_pool(name="agw", bufs=2)
    xo = tc.alloc_tile_pool(name="xo", bufs=1)
    wk = tc.alloc_tile_pool(name="wk", bufs=2)
    psH = tc.alloc_tile_pool(name="psH", bufs=1, space="PSUM")
    psS = tc.alloc_tile_pool(name="psS", bufs=1, space="PSUM")
    psN = tc.alloc_tile_pool(name="psN", bufs=2, space="PSUM")
    psT = tc.alloc_tile_pool(name="psT", bufs=2, space="PSUM")
    psY = tc.alloc_tile_pool(name="psY", bufs=1, space="PSUM")

    # ---- constants / weights ----
    identb = cst.tile([128, 128], BF16)
    masks.make_identity(nc, identb[:])

    xb = cst.tile([NH, 2 * NG], BF16)
    nc.sync.dma_start(xb[:], din["xb"].ap()[:, :])
    mmt = cst.tile([NH, 2 * NG], BF16)
    nc.sync.dma_start(mmt[:], din["mm"].ap()[:, :])
    w1s = cst.tile([NH, 2 * HC], BF16)
    nc.sync.dma_start(w1s[:], din["w1s"].ap()[:, :])
    w2s = cst.tile([128, 3 * HC], BF16)
    nc.sync.dma_start(w2s[:], din["w2s"].ap()[:, :])
    was1 = cst.tile([NH, 2 * 12], BF16)
    nc.sync.dma_start(was1[:], din["was1"].ap()[:, :])
    was2 = cst.tile([128, 3 * 12], BF16)
    nc.sync.dma_start(was2[:], din["was2"].ap()[:, :])
    gncol = cst.tile([128, 4], F32)
    nc.sync.dma_start(gncol[:], din["gncol"].ap()[:, :])
    gncol2 = cst.tile([128, 4], F32)
    nc.sync.dma_start(gncol2[:], din["gncol2"].ap()[:, :])
    head64 = cst.tile([C, 4], F32)
    nc.sync.dma_start(head64[:], din["head64"].ap()[:, :])
    lin2w = cst.tile([C, NCLS], F32)
    nc.sync.dma_start(lin2w[:], din["lin2w"].ap()[:, :])
    lin2b = cst.tile([NCLS, 1], F32)
    nc.sync.dma_start(lin2b[:], din["lin2b"].ap()[:, :])

    # lin1 weight stream: first two chunks begin now (overlap the layers)
    def lin1_chunk(i):
        t = lw.tile([128, JPC * C], BF16, tag="lin1")
        nc.sync.dma_start(t[:], din["lin1s"].ap()[:, i * JPC * C:(i + 1) * JPC * C])
        return t

    lin1_t = [lin1_chunk(0), lin1_chunk(1)]

    def layer(xBs, wts, wast, gcol, lay):
        """One GAT layer + elu + graphnorm for all 6 graphs.

        xBs: list of nkc channel-major input tiles [p, (g, n)] bf16.
        wts: [p, (kc, 384)] bf16; wast: [p, (kc, 12)] bf16.
        Returns 3 tiles [128, (g, n)] bf16 channel-major."""
        nkc = len(xBs)

        # h = W.T @ x -> hB [c(3x128), (g, n)] bf16
        hB = hp.tile([128, 3 * NG], BF16, tag="hB")
        for ck in range(3):
            for nb in range(3):
                h_ps = psH.tile([128, 456], F32, tag="hps")
                for kc in range(nkc):
                    nc.tensor.matmul(h_ps[:],
                                     wts[:, kc * HC + ck * 128: kc * HC + (ck + 1) * 128],
                                     xBs[kc][:, nb * 456:(nb + 1) * 456],
                                     start=(kc == 0), stop=(kc == nkc - 1))
                nc.scalar.copy(hB[:, ck * NG + nb * 456: ck * NG + (nb + 1) * 456], h_ps[:])

        # a12T [12, (g, n)] = was.T @ x
        a12T = att.tile([12, NG], BF16, tag="a12T")
        for nb in range(3):
            a_ps = psS.tile([12, 456], F32, tag="aps")
            for kc in range(nkc):
                nc.tensor.matmul(a_ps[:], wast[:, kc * 12:(kc + 1) * 12],
                                 xBs[kc][:, nb * 456:(nb + 1) * 456],
                                 start=(kc == 0), stop=(kc == nkc - 1))
            nc.vector.tensor_copy(a12T[:, nb * 456:(nb + 1) * 456], a_ps[:])

        # a2 rows relocated to partition 0 for partition_broadcast
        a2rs = att.tile([1, 6 * NG], BF16, tag="a2rs")
        for h in range(6):
            nc.sync.dma_start(a2rs[0:1, h * NG:(h + 1) * NG],
                              a12T[6 + h:7 + h, :])

        # a1A node-major [114, (sc, g, 6)] bf16
        a1A = att.tile([NH, 2 * GPC * 6], BF16, tag="a1A")
        for sc in range(2):
            a1_ps = psS.tile([NH, GPC * 6], BF16, tag="a1ps")
            for g in range(GPC):
                nc.tensor.transpose(
                    a1_ps[:, g * 6:(g + 1) * 6],
                    a12T[0:6, g * NPG + sc * NH: g * NPG + sc * NH + NH],
                    identb[0:6, 0:6])
            nc.vector.tensor_copy(a1A[:, sc * 36:(sc + 1) * 36], a1_ps[:])

        # hA65 node-major [114, (sc, g, h, 65)] bf16, 65th col = 1
        hA65 = att.tile([NH, 2 * GPC * 390], BF16, tag="hA65")
        for sc in range(2):
            for g in range(GPC):
                t_ps = psT.tile([128, 456], BF16, tag="tp")
                for ck in range(3):
                    nc.tensor.transpose(
                        t_ps[0:NH, ck * 128:(ck + 1) * 128],
                        hB[:, ck * NG + g * NPG + sc * NH: ck * NG + g * NPG + sc * NH + NH],
                        identb[:])
                dst = hA65[:, (sc * GPC + g) * 390:(sc * GPC + g + 1) * 390] \
                    .rearrange("p (h c) -> p h c", c=65)
                nc.scalar.copy(
                    dst[:, :, 0:64],
                    t_ps[0:NH, 0:HC].rearrange("p (h c) -> p h c", h=6))
                nc.gpsimd.memset(dst[:, :, 64:65], 1.0)

        # ---- attention + aggregation per graph ----
        x2B = []
        xef = []
        for ck in range(3):
            xt = xo.tile([128, NG], BF16, tag=f"x2B{ck}", name=f"x2B{ck}")
            x2B.append(xt)
            xf = wk.tile([128, NG], BF16, tag=f"xe{ck}", name=f"xe{ck}")
            xef.append(xf)
        s1t = wk.tile([128, 3 * GPC], F32, tag="s1t")
        s2t = wk.tile([128, 3 * GPC], F32, tag="s2t")
        x2ps = []
        for g in range(GPC):
            # bc[s, (h, d)] = a2[h, d] broadcast across partitions
            bc = scp.tile([NH, 6 * NPG], BF16, tag="bc")
            for h in range(6):
                nc.gpsimd.partition_broadcast(
                    bc[:, h * NPG:(h + 1) * NPG],
                    a2rs[0:1, h * NG + g * NPG: h * NG + (g + 1) * NPG])
            # per source-half: packed [114, (h, d)] tiles so DVE runs in 2x/4x
            zs = []
            for sc in range(2):
                mx = scp.tile([NH, 6 * NPG], BF16, tag=f"mx{sc}")
                nc.sync.dma_start(
                    mx[:],
                    mmt[:, (sc * GPC + g) * NPG:(sc * GPC + g + 1) * NPG]
                    .rearrange("p (h d) -> p h d", h=1).broadcast_to((NH, 6, NPG)))
                z = scp.tile([NH, 6 * NPG], BF16, tag=f"z{sc}")
                a1b = a1A[:, (sc * GPC + g) * 6:(sc * GPC + g + 1) * 6] \
                    .rearrange("p (h d) -> p h d", d=1).broadcast_to((NH, 6, NPG))
                nc.vector.tensor_tensor(
                    out=z[:].rearrange("p (h d) -> p h d", h=6),
                    in0=bc[:].rearrange("p (h d) -> p h d", h=6),
                    in1=a1b, op=Alu.add)
                nc.scalar.activation(z[:], z[:], Act.Prelu, alpha=0.2)
                nc.scalar.activation(z[:], z[:], Act.Exp)
                nc.vector.tensor_tensor(out=z[:], in0=z[:], in1=mx[:], op=Alu.mult)
                zs.append(z)

            # aggregation: psum [d(114), (h, 65)] per dc; col 64 = denominator
            x2p = agw.tile([NH, 2 * HC], BF16, tag="x2p")
            for dc in range(2):
                n_ps = psN.tile([NH, 390], F32, tag="nps")
                for h in range(6):
                    for sc in range(2):
                        nc.tensor.matmul(
                            n_ps[:, h * 65:(h + 1) * 65],
                            zs[sc][:, h * NPG + dc * NH: h * NPG + dc * NH + NH],
                            hA65[:, (sc * GPC + g) * 390 + h * 65:(sc * GPC + g) * 390 + (h + 1) * 65],
                            start=(sc == 0), stop=(sc == 1))
                rec = agw.tile([NH, 6], F32, tag="rec")
                nc.vector.reciprocal(
                    rec[:], n_ps[:].rearrange("p (h c) -> p h c", c=65)[:, :, 64:65]
                    .rearrange("p h c -> p (h c)"))
                nc.vector.tensor_tensor(
                    out=x2p[:, dc * HC:(dc + 1) * HC].rearrange("p (h c) -> p h c", h=6),
                    in0=n_ps[:].rearrange("p (h c) -> p h c", c=65)[:, :, 0:64],
                    in1=rec[:].rearrange("p (h c) -> p h c", c=1).broadcast_to((NH, 6, 64)),
                    op=Alu.mult)
            # transpose this graph's columns to channel-major right away
            for ck in range(3):
                tp = psT.tile([128, 456], BF16, tag="tp")
                for dc in range(2):
                    nc.tensor.transpose(
                        tp[:, dc * NH:(dc + 1) * NH],
                        x2p[:, dc * HC + ck * 128: dc * HC + (ck + 1) * 128],
                        identb[0:NH, 0:NH])
                nc.scalar.copy(x2B[ck][:, g * NPG:(g + 1) * NPG], tp[:, 0:NPG])
            # elu + per-graph stats for this graph's columns right away, so
            # the V/S work overlaps the remaining graphs' score pipeline
            for ck in range(3):
                xcol = x2B[ck][:, g * NPG:(g + 1) * NPG]
                m = wk.tile([128, NPG], BF16, tag="m")
                nc.vector.tensor_scalar_min(m[:], xcol, 0.0)
                e = wk.tile([128, NPG], BF16, tag="e")
                nc.scalar.activation(e[:], m[:], Act.Exp)
                xcl = xef[ck][:, g * NPG:(g + 1) * NPG]
                nc.vector.scalar_tensor_tensor(xcl, e[:], -1.0, xcol,
                                               op0=Alu.add, op1=Alu.max)
                nc.vector.tensor_reduce(s1t[:, ck * GPC + g: ck * GPC + g + 1],
                                        xcl, axis=mybir.AxisListType.X, op=Alu.add)
                sq = wk.tile([128, NPG], BF16, tag="sq")
                nc.vector.tensor_tensor(out=sq[:], in0=xcl, in1=xcl, op=Alu.mult)
                nc.vector.tensor_reduce(s2t[:, ck * GPC + g: ck * GPC + g + 1],
                                        sq[:], axis=mybir.AxisListType.X, op=Alu.add)
            x2ps.append(x2p)

        # ---- graphnorm scale/shift (stats already accumulated in-loop) ----
        mv = wk.tile([128, 2 * 3 * GPC], F32, tag="mv")   # mean cols | veps cols
        out_tiles = [None, None, None]

        def finish_ck(ck, isd_cols):
            gisd = wk.tile([128, GPC], F32, tag="gisd")
            nc.vector.tensor_scalar_mul(gisd[:], isd_cols, gcol[:, ck:ck + 1])
            tcol = wk.tile([128, GPC], F32, tag="tcol")
            nc.vector.tensor_tensor(out=tcol[:], in0=mv[:, ck * GPC:(ck + 1) * GPC],
                                    in1=gisd[:], op=Alu.mult)
            # out = xe * gisd - tcol   (gamma folded; beta==0)
            ot = xo.tile([128, NG], BF16, tag=f"xn{lay}{ck}", name=f"xn{lay}{ck}")
            ot3 = ot[:].rearrange("p (g n) -> p g n", g=GPC)
            nc.gpsimd.tensor_tensor(out=ot3,
                                    in0=xef[ck][:].rearrange("p (g n) -> p g n", g=GPC),
                                    in1=gisd[:].rearrange("p (g n) -> p g n", n=1)
                                    .broadcast_to((128, GPC, NPG)),
                                    op=Alu.mult)
            nc.gpsimd.tensor_tensor(out=ot3, in0=ot3,
                                    in1=tcol[:].rearrange("p (g n) -> p g n", n=1)
                                    .broadcast_to((128, GPC, NPG)),
                                    op=Alu.subtract)
            out_tiles[ck] = ot

        mean18 = mv[:, 0:3 * GPC]
        nc.vector.tensor_scalar_mul(mean18, s1t[:], 1.0 / NPG)
        msq = wk.tile([128, 3 * GPC], F32, tag="msq")
        nc.vector.tensor_tensor(out=msq[:], in0=mean18, in1=mean18, op=Alu.mult)
        veps18 = mv[:, 3 * GPC:6 * GPC]
        nc.vector.scalar_tensor_tensor(veps18, s2t[:], 1.0 / NPG, msq[:],
                                       op0=Alu.mult, op1=Alu.subtract)
        # one Ln + one Exp for all three ck chunks
        lnv = wk.tile([128, 3 * GPC], F32, tag="lnv")
        nc.vector.tensor_scalar_add(lnv[:], veps18, 1e-5)
        nc.scalar.activation(lnv[:], lnv[:], Act.Ln)
        isd = wk.tile([128, 3 * GPC], F32, tag="isd")
        nc.scalar.activation(isd[:], lnv[:], Act.Exp, scale=-0.5)
        for ck in range(3):
            finish_ck(ck, isd[:, ck * GPC:(ck + 1) * GPC])
        return out_tiles

    x2 = layer([xb[:, 0:NG], xb[:, NG:2 * NG]], w1s, was1, gncol, 0)
    x3 = layer([x2[0][:], x2[1][:], x2[2][:]], w2s, was2, gncol2, 1)

    # remaining lin1 weight chunks (double-buffered against GEMV consumption)
    for i in range(2, NLCH):
        lin1_t.append(lin1_chunk(i))

    # ---- lin1 GEMV: 684 k=128 chunks, alternating PE column tiles so the
    # next chunk's weight load overlaps the current chunk's matmul ----
    y_ps = psY.tile([128, GPC], F32, tag="y")
    for i in range(NLCH):
        lt = lin1_t[i]
        for jj in range(JPC):
            jc = i * JPC + jj
            ck, n = jc // NPG, jc % NPG
            x3r = x3[ck][:].rearrange("p (g n) -> p n g", g=GPC)
            nc.tensor.matmul(y_ps[0:C, :], lt[:, jj * C:(jj + 1) * C],
                             x3r[:, n, :],
                             start=(jc == 0), stop=(jc == NJ1 - 1))

    # ---- head: +b, elu, bn, lin2 ----
    yb = wk.tile([C, GPC], F32, tag="yb")
    nc.vector.tensor_scalar_add(yb[:], y_ps[0:C, :], head64[:, 0:1])
    m2 = wk.tile([C, GPC], F32, tag="m2")
    nc.vector.tensor_scalar_min(m2[:], yb[:], 0.0)
    e2 = wk.tile([C, GPC], F32, tag="e2")
    nc.scalar.activation(e2[:], m2[:], Act.Exp)
    ye = wk.tile([C, GPC], F32, tag="ye")
    nc.vector.scalar_tensor_tensor(ye[:], e2[:], -1.0, yb[:], op0=Alu.add, op1=Alu.max)
    yn = wk.tile([C, GPC], F32, tag="yn")
    nc.vector.scalar_tensor_tensor(yn[:], ye[:], head64[:, 1:2],
                                   head64[:, 2:3].broadcast_to((C, GPC)),
                                   op0=Alu.mult, op1=Alu.add)
    o_ps = psY.tile([128, GPC], F32, tag="y")
    nc.tensor.matmul(o_ps[0:NCLS, :], lin2w[:], yn[:], start=True, stop=True)
    ob = wk.tile([NCLS, GPC], F32, tag="ob")
    nc.vector.tensor_scalar_add(ob[:], o_ps[0:NCLS, :], lin2b[:])
    nc.sync.dma_start(out_d.ap()[:, :], ob[:])

    for p in (psY, psT, psN, psS, psH, wk, xo, agw, scp, att, hp, lw, cst):
        p.release()


def _host_prep(inputs):
    """Build per-core input maps (sharding / relayout / dtype prep)."""
    import ml_dtypes
    x = np.asarray(inputs["x"], np.float32)
    ei = np.asarray(inputs["edge_index"])
    src, dst = np.asarray(ei[0], np.int64), np.asarray(ei[1], np.int64)

    # multiplicity matrices M[g, s, d] (+ self loops)
    g_of = src // NPG
    sl = src - g_of * NPG
    dl = dst - (dst // NPG) * NPG
    flat = g_of * (NPG * NPG) + sl * NPG + dl
    Mall = np.bincount(flat, minlength=B * NPG * NPG).astype(np.float32).reshape(B, NPG, NPG)
    Mall[:, np.arange(NPG), np.arange(NPG)] += 1.0

    xg = x.reshape(B, NPG, F_IN)

    def mk_asad(a_s, a_d):
        a_s = np.asarray(a_s, np.float32)
        a_d = np.asarray(a_d, np.float32)
        out = np.zeros((HC, 12), np.float32)
        for h in range(H):
            out[h * C:(h + 1) * C, h] = a_s[h]
            out[h * C:(h + 1) * C, 6 + h] = a_d[h]
        return out

    w1 = np.asarray(inputs["w1"], np.float32)
    w2 = np.asarray(inputs["w2"], np.float32)
    was1 = w1 @ mk_asad(inputs["as1"], inputs["ad1"])   # [228, 12]
    was2 = w2 @ mk_asad(inputs["as2"], inputs["ad2"])   # [384, 12]

    # kernel folds assume zero biases / unit mean-scale (true for this model)
    for nm in ("b1", "b2", "gn1_b", "gn2_b"):
        assert np.abs(np.asarray(inputs[nm])).max() == 0.0, f"{nm} nonzero"
    for nm in ("gn1_ms", "gn2_ms"):
        assert np.abs(np.asarray(inputs[nm]) - 1.0).max() == 0.0, f"{nm} != 1"

    bn_w = np.asarray(inputs["bn_w"], np.float64)
    bn_b = np.asarray(inputs["bn_b"], np.float64)
    bn_rm = np.asarray(inputs["bn_rm"], np.float64)
    bn_rv = np.asarray(inputs["bn_rv"], np.float64)
    bn_sc = bn_w / np.sqrt(bn_rv + 1e-5)
    bn_sh = bn_b - bn_rm * bn_sc
    head64 = np.stack([np.asarray(inputs["lin1_b"], np.float64),
                       bn_sc, bn_sh, np.zeros((C,))], axis=1).astype(np.float32)

    # lin1 reorder: rows j=(n*384 + ck*128 + p) -> chunks (ck, n) of k=128
    lwt = np.asarray(inputs["lin1_w"], np.float32).reshape(NPG, 3, 128, C)
    lin1s = np.ascontiguousarray(lwt.transpose(2, 1, 0, 3)).reshape(128, NJ1 * C) \
        .astype(ml_dtypes.bfloat16)

    def cm(a):
        """[g, n, f] -> [114 (f-part), (fc, g, n)] bf16 channel-major."""
        gg, nn, ff = a.shape
        nkc = ff // NH
        t = a.transpose(2, 0, 1).reshape(nkc, NH, gg, nn).transpose(1, 0, 2, 3)
        return np.ascontiguousarray(t).reshape(NH, nkc * gg * nn).astype(ml_dtypes.bfloat16)

    gnc1 = np.zeros((128, 4), np.float32)
    gnc2 = np.zeros((128, 4), np.float32)
    gnc1[:, 0:3] = np.asarray(inputs["gn1_w"], np.float32).reshape(3, 128).T
    gnc2[:, 0:3] = np.asarray(inputs["gn2_w"], np.float32).reshape(3, 128).T

    shared = dict(
        w1s=np.ascontiguousarray(
            w1.reshape(2, NH, HC).transpose(1, 0, 2)).reshape(NH, 2 * HC)
            .astype(ml_dtypes.bfloat16),
        w2s=np.ascontiguousarray(
            w2.reshape(3, 128, HC).transpose(1, 0, 2)).reshape(128, 3 * HC)
            .astype(ml_dtypes.bfloat16),
        was1=np.ascontiguousarray(
            was1.reshape(2, NH, 12).transpose(1, 0, 2)).reshape(NH, 24)
            .astype(ml_dtypes.bfloat16),
        was2=np.ascontiguousarray(
            was2.reshape(3, 128, 12).transpose(1, 0, 2)).reshape(128, 36)
            .astype(ml_dtypes.bfloat16),
        gncol=gnc1, gncol2=gnc2,
        lin1s=lin1s, head64=head64,
        lin2w=np.asarray(inputs["lin2_w"], np.float32),
        lin2b=np.asarray(inputs["lin2_b"], np.float32).reshape(NCLS, 1),
    )

    in_maps = []
    for core in range(NCORES):
        gs = slice(core * GPC, (core + 1) * GPC)
        m = dict(shared)
        m["xb"] = cm(xg[gs])                           # [114, (fc, g, n)]
        m["mm"] = cm(Mall[gs].transpose(0, 2, 1))      # [114 (s), (sc, g, d)]
        in_maps.append(m)
    return in_maps


_cached_nc = None


def kernel(**inputs):
    global _cached_nc
    in_maps = _host_prep(inputs)
    if _cached_nc is None:
        _cached_nc = _build_program()
    nc = _cached_nc
    res = bass_utils.run_bass_kernel_spmd(nc, in_maps, core_ids=list(range(NCORES)))
    _last_results["exec_time_ns"] = res.exec_time_ns
    _last_results["res"] = res
    out = np.zeros((B, NCLS), np.float32)
    for core in range(NCORES):
        o = res.results[core]["out"]          # [2, 6]
        out[core * GPC:(core + 1) * GPC, :] = o.T
    return out



# revision 5
# speedup vs baseline: 1.0723x; 1.0723x over previous
"""GAT (2-layer, 6-head) + GraphNorm + readout MLP on 8 Trainium2 cores.

Sharding: graph-level data parallelism. 48 fixed-size graphs (228 nodes,
edges never cross graphs) -> 6 graphs per core. Weights replicated.

v2 redesign vs the per-graph baseline:
  - All 6 graphs batched per stage; channel-major [c, (g, n)] primary layout.
  - Dense attention scores z[s,d] built with GpSimd partition-broadcast of
    the a2 rows + one wide DVE add per graph ([114, 2*6*228] bf16 tiles,
    both source-halves per instruction), lrelu on DVE, exp on Scalar
    (single activation table: exp/ln/relu/copy), multiplicity mask on GpSimd.
  - Attention logits a1/a2 computed straight from the layer input with
    host-folded was = W @ [as|ad].
  - Aggregation: dest-partition matmuls with a fused ones-column so the
    softmax denominator falls out of the same PSUM tile.
  - lin1 readout: weights host-reordered to (ck, n, p) chunks of k=128 so
    the GEMV consumes the channel-major layer-2 output directly; weights
    streamed in 4 double-buffered DMA chunks overlapping the layer phase.

kernel(**inputs) -> np.ndarray [48, 2] float32.
"""
import sys
sys.path.insert(0, '/opt/trn_rl_repo')

import numpy as np

import concourse.bass as bass
import concourse.bacc as bacc
import concourse.mybir as mybir
import concourse.tile as tile
from concourse import masks
from concourse import bass_utils

F32 = mybir.dt.float32
BF16 = mybir.dt.bfloat16
Alu = mybir.AluOpType
Act = mybir.ActivationFunctionType

H, C = 6, 64
HC = 384
NPG = 228          # nodes per graph
B = 48             # graphs
GPC = 6            # graphs per core
NCORES = 8
F_IN = 228
NH = 114           # node half-chunk
NCLS = 2
NG = GPC * NPG     # 1368 node-columns per core
NJ1 = 3 * NPG      # 684 lin1 k-chunks of 128
NLCH = 6           # lin1 weight stream chunks (even JPC so FWL pairs don't split)
JPC = NJ1 // NLCH  # 114 chunks per stream piece

_last_results = {"exec_time_ns": None}


def _ensure_axon_hooks():
    """Make BASS_TRACE-driven NTFF profiling under axon degrade gracefully."""
    try:
        import antenv.axon_hooks  # noqa: F401
        return
    except ImportError:
        pass
    import types
    try:
        import antenv
    except ImportError:
        return
    mod = types.ModuleType("antenv.axon_hooks")
    holder = {"hook": None}
    mod.set_axon_ntff_profile_hook = lambda h: holder.__setitem__("hook", h)
    mod.get_axon_ntff_profile_hook = lambda: holder["hook"]
    sys.modules["antenv.axon_hooks"] = mod
    antenv.axon_hooks = mod
    try:
        from trn_agent_boot.trn_boot import _ntff_profile_via_ctypes
        hook = _ntff_profile_via_ctypes('/opt/axon/libaxon_pjrt.so')
        if hook is not None:
            mod.set_axon_ntff_profile_hook(hook)
    except Exception:
        pass
    _orig_upload = bass_utils.upload_artifacts

    def _safe_upload(tmpdir):
        try:
            return _orig_upload(tmpdir)
        except Exception:
            return "local://" + str(tmpdir)

    bass_utils.upload_artifacts = _safe_upload


_ensure_axon_hooks()


def _build_program():
    nc = bacc.Bacc("TRN2", target_bir_lowering=False, debug=False)

    dt_in = {}

    def din(name, shape, dtype=F32):
        t = nc.dram_tensor(name, shape, dtype, kind="ExternalInput")
        dt_in[name] = t
        return t

    din("xb", [NH, 2 * NG], BF16)            # x chan-major [p, (fc, g, n)]
    din("mm", [NH, 2 * NG], BF16)            # multiplicity+I [p, (sc, g, d)]
    din("w1s", [NH, 2 * HC], BF16)           # W1 [p, (fc, 384)]
    din("w2s", [128, 3 * HC], BF16)          # W2 [p, (kc, 384)]
    din("was1", [NH, 2 * 12], BF16)          # W1@[as|ad] [p, (fc, 12)]
    din("was2", [128, 3 * 12], BF16)
    din("gncol", [128, 4], F32)              # graphnorm gamma, col ck
    din("gncol2", [128, 4], F32)
    din("lin1s", [128, NJ1 * C], BF16)       # lin1_w reordered (p, (ck, n, 64))
    din("head64", [C, 4], F32)               # cols: lin1_b, bn_scale, bn_shift
    din("lin2w", [C, NCLS], F32)
    din("lin2b", [NCLS, 1], F32)

    out_d = nc.dram_tensor("out", [NCLS, GPC], F32, kind="ExternalOutput")

    with tile.TileContext(nc) as tc:
        _emit(tc, dt_in, out_d)

    nc.finalize()
    return nc


def _emit(tc, din, out_d):
    nc = tc.nc

    cst = tc.alloc_tile_pool(name="cst", bufs=1)
    lw = tc.alloc_tile_pool(name="lw", bufs=2)
    hp = tc.alloc_tile_pool(name="hp", bufs=1)
    att = tc.alloc_tile_pool(name="att", bufs=1)
    scp = tc.alloc_tile_pool(name="scp", bufs=2)
    agw = tc.alloc_tile_pool(name="agw", bufs=2)
    xo = tc.alloc_tile_pool(name="xo", bufs=1)
    wk = tc.alloc_tile_pool(name="wk", bufs=2)
    psH = tc.alloc_tile_pool(name="psH", bufs=1, space="PSUM")
    psS = tc.alloc_tile_pool(name="psS", bufs=1, space="PSUM")
    psN = tc.alloc_tile_pool(name="psN", bufs=2, space="PSUM")
    psT = tc.alloc_tile_pool(name="psT", bufs=2, space="PSUM")
    psY = tc.alloc_tile_pool(name="psY", bufs=1, space="PSUM")

    # ---- constants / weights ----
    identb = cst.tile([128, 128], BF16)
    masks.make_identity(nc, identb[:])

    xb = cst.tile([NH, 2 * NG], BF16)
    nc.sync.dma_start(xb[:], din["xb"].ap()[:, :])
    mmt = cst.tile([NH, 2 * NG], BF16)
    nc.sync.dma_start(mmt[:], din["mm"].ap()[:, :])
    w1s = cst.tile([NH, 2 * HC], BF16)
    nc.sync.dma_start(w1s[:], din["w1s"].ap()[:, :])
    w2s = cst.tile([128, 3 * HC], BF16)
    nc.sync.dma_start(w2s[:], din["w2s"].ap()[:, :])
    was1 = cst.tile([NH, 2 * 12], BF16)
    nc.sync.dma_start(was1[:], din["was1"].ap()[:, :])
    was2 = cst.tile([128, 3 * 12], BF16)
    nc.sync.dma_start(was2[:], din["was2"].ap()[:, :])
    gncol = cst.tile([128, 4], F32)
    nc.sync.dma_start(gncol[:], din["gncol"].ap()[:, :])
    gncol2 = cst.tile([128, 4], F32)
    nc.sync.dma_start(gncol2[:], din["gncol2"].ap()[:, :])
    head64 = cst.tile([C, 4], F32)
    nc.sync.dma_start(head64[:], din["head64"].ap()[:, :])
    lin2w = cst.tile([C, NCLS], F32)
    nc.sync.dma_start(lin2w[:], din["lin2w"].ap()[:, :])
    lin2b = cst.tile([NCLS, 1], F32)
    nc.sync.dma_start(lin2b[:], din["lin2b"].ap()[:, :])

    # lin1 weight stream: first two chunks begin now (overlap the layers)
    def lin1_chunk(i):
        t = lw.tile([128, JPC * C], BF16, tag="lin1")
        nc.sync.dma_start(t[:], din["lin1s"].ap()[:, i * JPC * C:(i + 1) * JPC * C])
        return t

    lin1_t = [lin1_chunk(0), lin1_chunk(1)]

    def layer(xBs, wts, wast, gcol, lay):
        """One GAT layer + elu + graphnorm for all 6 graphs.

        xBs: list of nkc channel-major input tiles [p, (g, n)] bf16.
        wts: [p, (kc, 384)] bf16; wast: [p, (kc, 12)] bf16.
        Returns 3 tiles [128, (g, n)] bf16 channel-major."""
        nkc = len(xBs)

        # h = W.T @ x -> hB [c(3x128), (g, n)] bf16
        hB = hp.tile([128, 3 * NG], BF16, tag="hB")
        for ck in range(3):
            for nb in range(3):
                h_ps = psH.tile([128, 456], F32, tag="hps")
                for kc in range(nkc):
                    nc.tensor.matmul(h_ps[:],
                                     wts[:, kc * HC + ck * 128: kc * HC + (ck + 1) * 128],
                                     xBs[kc][:, nb * 456:(nb + 1) * 456],
                                     start=(kc == 0), stop=(kc == nkc - 1))
                nc.scalar.copy(hB[:, ck * NG + nb * 456: ck * NG + (nb + 1) * 456], h_ps[:])

        # a12T [12, (g, n)] = was.T @ x
        a12T = att.tile([12, NG], BF16, tag="a12T")
        for nb in range(3):
            a_ps = psS.tile([12, 456], F32, tag="aps")
            for kc in range(nkc):
                nc.tensor.matmul(a_ps[:], wast[:, kc * 12:(kc + 1) * 12],
                                 xBs[kc][:, nb * 456:(nb + 1) * 456],
                                 start=(kc == 0), stop=(kc == nkc - 1))
            nc.vector.tensor_copy(a12T[:, nb * 456:(nb + 1) * 456], a_ps[:])

        # a2 rows relocated to partition 0 for partition_broadcast
        a2rs = att.tile([1, 6 * NG], BF16, tag="a2rs")
        for h in range(6):
            nc.sync.dma_start(a2rs[0:1, h * NG:(h + 1) * NG],
                              a12T[6 + h:7 + h, :])

        # a1A node-major [114, (sc, g, 6)] bf16
        a1A = att.tile([NH, 2 * GPC * 6], BF16, tag="a1A")
        for sc in range(2):
            a1_ps = psS.tile([NH, GPC * 6], BF16, tag="a1ps")
            for g in range(GPC):
                nc.tensor.transpose(
                    a1_ps[:, g * 6:(g + 1) * 6],
                    a12T[0:6, g * NPG + sc * NH: g * NPG + sc * NH + NH],
                    identb[0:6, 0:6])
            nc.vector.tensor_copy(a1A[:, sc * 36:(sc + 1) * 36], a1_ps[:])

        # hA65 node-major [114, (sc, g, h, 65)] bf16, 65th col = 1
        hA65 = att.tile([NH, 2 * GPC * 390], BF16, tag="hA65")
        for sc in range(2):
            for g in range(GPC):
                t_ps = psT.tile([128, 456], BF16, tag="tp")
                for ck in range(3):
                    nc.tensor.transpose(
                        t_ps[0:NH, ck * 128:(ck + 1) * 128],
                        hB[:, ck * NG + g * NPG + sc * NH: ck * NG + g * NPG + sc * NH + NH],
                        identb[:])
                dst = hA65[:, (sc * GPC + g) * 390:(sc * GPC + g + 1) * 390] \
                    .rearrange("p (h c) -> p h c", c=65)
                nc.scalar.copy(
                    dst[:, :, 0:64],
                    t_ps[0:NH, 0:HC].rearrange("p (h c) -> p h c", h=6))
                nc.gpsimd.memset(dst[:, :, 64:65], 1.0)

        # ---- attention + aggregation per graph ----
        x2B = []
        xef = []
        for ck in range(3):
            xt = xo.tile([128, NG], BF16, tag=f"x2B{ck}", name=f"x2B{ck}")
            x2B.append(xt)
            xf = wk.tile([128, NG], BF16, tag=f"xe{ck}", name=f"xe{ck}")
            xef.append(xf)
        s1t = wk.tile([128, 3 * GPC], F32, tag="s1t")
        s2t = wk.tile([128, 3 * GPC], F32, tag="s2t")
        x2ps = []
        for g in range(GPC):
            # bc[s, (h, d)] = a2[h, d] broadcast across partitions
            bc = scp.tile([NH, 6 * NPG], BF16, tag="bc")
            for h in range(6):
                nc.gpsimd.partition_broadcast(
                    bc[:, h * NPG:(h + 1) * NPG],
                    a2rs[0:1, h * NG + g * NPG: h * NG + (g + 1) * NPG])
            # per source-half: packed [114, (h, d)] tiles so DVE runs in 2x/4x
            zs = []
            for sc in range(2):
                mx = scp.tile([NH, 6 * NPG], BF16, tag=f"mx{sc}")
                nc.sync.dma_start(
                    mx[:],
                    mmt[:, (sc * GPC + g) * NPG:(sc * GPC + g + 1) * NPG]
                    .rearrange("p (h d) -> p h d", h=1).broadcast_to((NH, 6, NPG)))
                z = scp.tile([NH, 6 * NPG], BF16, tag=f"z{sc}")
                a1b = a1A[:, (sc * GPC + g) * 6:(sc * GPC + g + 1) * 6] \
                    .rearrange("p (h d) -> p h d", d=1).broadcast_to((NH, 6, NPG))
                nc.vector.tensor_tensor(
                    out=z[:].rearrange("p (h d) -> p h d", h=6),
                    in0=bc[:].rearrange("p (h d) -> p h d", h=6),
                    in1=a1b, op=Alu.add)
                nc.scalar.activation(z[:], z[:], Act.Prelu, alpha=0.2)
                nc.scalar.activation(z[:], z[:], Act.Exp)
                nc.vector.tensor_tensor(out=z[:], in0=z[:], in1=mx[:], op=Alu.mult)
                zs.append(z)

            # aggregation: psum [d(114), (h, 65)] per dc; col 64 = denominator
            x2p = agw.tile([NH, 2 * HC], BF16, tag="x2p")
            for dc in range(2):
                n_ps = psN.tile([NH, 390], F32, tag="nps")
                for h in range(6):
                    for sc in range(2):
                        nc.tensor.matmul(
                            n_ps[:, h * 65:(h + 1) * 65],
                            zs[sc][:, h * NPG + dc * NH: h * NPG + dc * NH + NH],
                            hA65[:, (sc * GPC + g) * 390 + h * 65:(sc * GPC + g) * 390 + (h + 1) * 65],
                            start=(sc == 0), stop=(sc == 1))
                rec = agw.tile([NH, 6], F32, tag="rec")
                nc.vector.reciprocal(
                    rec[:], n_ps[:].rearrange("p (h c) -> p h c", c=65)[:, :, 64:65]
                    .rearrange("p h c -> p (h c)"))
                nc.vector.tensor_tensor(
                    out=x2p[:, dc * HC:(dc + 1) * HC].rearrange("p (h c) -> p h c", h=6),
                    in0=n_ps[:].rearrange("p (h c) -> p h c", c=65)[:, :, 0:64],
                    in1=rec[:].rearrange("p (h c) -> p h c", c=1).broadcast_to((NH, 6, 64)),
                    op=Alu.mult)
            # transpose this graph's columns to channel-major right away
            for ck in range(3):
                tp = psT.tile([128, 456], BF16, tag="tp")
                for dc in range(2):
                    nc.tensor.transpose(
                        tp[:, dc * NH:(dc + 1) * NH],
                        x2p[:, dc * HC + ck * 128: dc * HC + (ck + 1) * 128],
                        identb[0:NH, 0:NH])
                nc.scalar.copy(x2B[ck][:, g * NPG:(g + 1) * NPG], tp[:, 0:NPG])
            # elu + per-graph stats for this graph's columns right away, so
            # the V/S work overlaps the remaining graphs' score pipeline
            for ck in range(3):
                xcol = x2B[ck][:, g * NPG:(g + 1) * NPG]
                m = wk.tile([128, NPG], BF16, tag="m")
                nc.vector.tensor_scalar_min(m[:], xcol, 0.0)
                e = wk.tile([128, NPG], BF16, tag="e")
                nc.scalar.activation(e[:], m[:], Act.Exp)
                xcl = xef[ck][:, g * NPG:(g + 1) * NPG]
                nc.vector.scalar_tensor_tensor(xcl, e[:], -1.0, xcol,
                                               op0=Alu.add, op1=Alu.max)
                nc.vector.tensor_reduce(s1t[:, ck * GPC + g: ck * GPC + g + 1],
                                        xcl, axis=mybir.AxisListType.X, op=Alu.add)
                sq = wk.tile([128, NPG], BF16, tag="sq")
                nc.vector.tensor_tensor(out=sq[:], in0=xcl, in1=xcl, op=Alu.mult)
                nc.vector.tensor_reduce(s2t[:, ck * GPC + g: ck * GPC + g + 1],
                                        sq[:], axis=mybir.AxisListType.X, op=Alu.add)
            x2ps.append(x2p)

        # ---- graphnorm scale/shift (stats already accumulated in-loop) ----
        mv = wk.tile([128, 2 * 3 * GPC], F32, tag="mv")   # mean cols | veps cols
        out_tiles = [None, None, None]

        def finish_ck(ck, isd_cols):
            gisd = wk.tile([128, GPC], F32, tag="gisd")
            nc.vector.tensor_scalar_mul(gisd[:], isd_cols, gcol[:, ck:ck + 1])
            tcol = wk.tile([128, GPC], F32, tag="tcol")
            nc.vector.tensor_tensor(out=tcol[:], in0=mv[:, ck * GPC:(ck + 1) * GPC],
                                    in1=gisd[:], op=Alu.mult)
            # out = xe * gisd - tcol   (gamma folded; beta==0)
            ot = xo.tile([128, NG], BF16, tag=f"xn{lay}{ck}", name=f"xn{lay}{ck}")
            ot3 = ot[:].rearrange("p (g n) -> p g n", g=GPC)
            nc.gpsimd.tensor_tensor(out=ot3,
                                    in0=xef[ck][:].rearrange("p (g n) -> p g n", g=GPC),
                                    in1=gisd[:].rearrange("p (g n) -> p g n", n=1)
                                    .broadcast_to((128, GPC, NPG)),
                                    op=Alu.mult)
            nc.gpsimd.tensor_tensor(out=ot3, in0=ot3,
                                    in1=tcol[:].rearrange("p (g n) -> p g n", n=1)
                                    .broadcast_to((128, GPC, NPG)),
                                    op=Alu.subtract)
            out_tiles[ck] = ot

        mean18 = mv[:, 0:3 * GPC]
        nc.vector.tensor_scalar_mul(mean18, s1t[:], 1.0 / NPG)
        msq = wk.tile([128, 3 * GPC], F32, tag="msq")
        nc.vector.tensor_tensor(out=msq[:], in0=mean18, in1=mean18, op=Alu.mult)
        veps18 = mv[:, 3 * GPC:6 * GPC]
        nc.vector.scalar_tensor_tensor(veps18, s2t[:], 1.0 / NPG, msq[:],
                                       op0=Alu.mult, op1=Alu.subtract)
        # one Ln + one Exp for all three ck chunks
        lnv = wk.tile([128, 3 * GPC], F32, tag="lnv")
        nc.vector.tensor_scalar_add(lnv[:], veps18, 1e-5)
        nc.scalar.activation(lnv[:], lnv[:], Act.Ln)
        isd = wk.tile([128, 3 * GPC], F32, tag="isd")
        nc.scalar.activation(isd[:], lnv[:], Act.Exp, scale=-0.5)
        for ck in range(3):
            finish_ck(ck, isd[:, ck * GPC:(ck + 1) * GPC])
        return out_tiles

    x2 = layer([xb[:, 0:NG], xb[:, NG:2 * NG]], w1s, was1, gncol, 0)
    x3 = layer([x2[0][:], x2[1][:], x2[2][:]], w2s, was2, gncol2, 1)

    # remaining lin1 weight chunks (double-buffered against GEMV consumption)
    for i in range(2, NLCH):
        lin1_t.append(lin1_chunk(i))

    # ---- lin1 GEMV: 684 k=128 chunks processed in PAIRS. Each pair loads a
    # full [128, 128] stationary tile (two adjacent nodes' weight chunks side
    # by side -> FWL-eligible) against a [128, 2, 6] moving slice. Diagonal
    # blocks of the [128, 12] psum hold the real partials; off-diagonal blocks
    # accumulate ignored cross terms. ----
    y_ps = psY.tile([128, 2 * GPC], F32, tag="y")
    for i in range(NLCH):
        lt = lin1_t[i]
        for jj in range(0, JPC, 2):
            jc = i * JPC + jj
            ck, n = jc // NPG, jc % NPG
            x3r = x3[ck][:].rearrange("p (g n) -> p n g", g=GPC)
            nc.tensor.matmul(y_ps[:], lt[:, jj * C:(jj + 2) * C],
                             x3r[:, n:n + 2, :],
                             start=(jc == 0), stop=(jc == NJ1 - 2))

    # fold: y = y_ps[0:64, 0:6] + y_ps[64:128, 6:12] (partition shift via DMA)
    yhi = wk.tile([128, GPC], F32, tag="yhi")
    nc.scalar.copy(yhi[64:128, :], y_ps[64:128, GPC:2 * GPC])
    ylo = wk.tile([C, GPC], F32, tag="ylo")
    nc.sync.dma_start(ylo[:], yhi[64:128, :])

    # ---- head: +b, elu, bn, lin2 ----
    yb = wk.tile([C, GPC], F32, tag="yb")
    nc.vector.scalar_tensor_tensor(yb[:], y_ps[0:C, 0:GPC], head64[:, 0:1],
                                   ylo[:], op0=Alu.add, op1=Alu.add)
    m2 = wk.tile([C, GPC], F32, tag="m2")
    nc.vector.tensor_scalar_min(m2[:], yb[:], 0.0)
    e2 = wk.tile([C, GPC], F32, tag="e2")
    nc.scalar.activation(e2[:], m2[:], Act.Exp)
    ye = wk.tile([C, GPC], F32, tag="ye")
    nc.vector.scalar_tensor_tensor(ye[:], e2[:], -1.0, yb[:], op0=Alu.add, op1=Alu.max)
    yn = wk.tile([C, GPC], F32, tag="yn")
    nc.vector.scalar_tensor_tensor(yn[:], ye[:], head64[:, 1:2],
                                   head64[:, 2:3].broadcast_to((C, GPC)),
                                   op0=Alu.mult, op1=Alu.add)
    o_ps = psY.tile([128, 2 * GPC], F32, tag="y")
    nc.tensor.matmul(o_ps[0:NCLS, 0:GPC], lin2w[:], yn[:], start=True, stop=True)
    ob = wk.tile([NCLS, GPC], F32, tag="ob")
    nc.vector.tensor_scalar_add(ob[:], o_ps[0:NCLS, 0:GPC], lin2b[:])
    nc.sync.dma_start(out_d.ap()[:, :], ob[:])

    for p in (psY, psT, psN, psS, psH, wk, xo, agw, scp, att, hp, lw, cst):
        p.release()


def _host_prep(inputs):
    """Build per-core input maps (sharding / relayout / dtype prep)."""
    import ml_dtypes
    x = np.asarray(inputs["x"], np.float32)
    ei = np.asarray(inputs["edge_index"])
    src, dst = np.asarray(ei[0], np.int64), np.asarray(ei[1], np.int64)

    # multiplicity matrices M[g, s, d] (+ self loops)
    g_of = src // NPG
    sl = src - g_of * NPG
    dl = dst - (dst // NPG) * NPG
    flat = g_of * (NPG * NPG) + sl * NPG + dl
    Mall = np.bincount(flat, minlength=B * NPG * NPG).astype(np.float32).reshape(B, NPG, NPG)
    Mall[:, np.arange(NPG), np.arange(NPG)] += 1.0

    xg = x.reshape(B, NPG, F_IN)

    def mk_asad(a_s, a_d):
        a_s = np.asarray(a_s, np.float32)
        a_d = np.asarray(a_d, np.float32)
        out = np.zeros((HC, 12), np.float32)
        for h in range(H):
            out[h * C:(h + 1) * C, h] = a_s[h]
            out[h * C:(h + 1) * C, 6 + h] = a_d[h]
        return out

    w1 = np.asarray(inputs["w1"], np.float32)
    w2 = np.asarray(inputs["w2"], np.float32)
    was1 = w1 @ mk_asad(inputs["as1"], inputs["ad1"])   # [228, 12]
    was2 = w2 @ mk_asad(inputs["as2"], inputs["ad2"])   # [384, 12]

    # kernel folds assume zero biases / unit mean-scale (true for this model)
    for nm in ("b1", "b2", "gn1_b", "gn2_b"):
        assert np.abs(np.asarray(inputs[nm])).max() == 0.0, f"{nm} nonzero"
    for nm in ("gn1_ms", "gn2_ms"):
        assert np.abs(np.asarray(inputs[nm]) - 1.0).max() == 0.0, f"{nm} != 1"

    bn_w = np.asarray(inputs["bn_w"], np.float64)
    bn_b = np.asarray(inputs["bn_b"], np.float64)
    bn_rm = np.asarray(inputs["bn_rm"], np.float64)
    bn_rv = np.asarray(inputs["bn_rv"], np.float64)
    bn_sc = bn_w / np.sqrt(bn_rv + 1e-5)
    bn_sh = bn_b - bn_rm * bn_sc
    head64 = np.stack([np.asarray(inputs["lin1_b"], np.float64),
                       bn_sc, bn_sh, np.zeros((C,))], axis=1).astype(np.float32)

    # lin1 reorder: rows j=(n*384 + ck*128 + p) -> chunks (ck, n) of k=128
    lwt = np.asarray(inputs["lin1_w"], np.float32).reshape(NPG, 3, 128, C)
    lin1s = np.ascontiguousarray(lwt.transpose(2, 1, 0, 3)).reshape(128, NJ1 * C) \
        .astype(ml_dtypes.bfloat16)

    def cm(a):
        """[g, n, f] -> [114 (f-part), (fc, g, n)] bf16 channel-major."""
        gg, nn, ff = a.shape
        nkc = ff // NH
        t = a.transpose(2, 0, 1).reshape(nkc, NH, gg, nn).transpose(1, 0, 2, 3)
        return np.ascontiguousarray(t).reshape(NH, nkc * gg * nn).astype(ml_dtypes.bfloat16)

    gnc1 = np.zeros((128, 4), np.float32)
    gnc2 = np.zeros((128, 4), np.float32)
    gnc1[:, 0:3] = np.asarray(inputs["gn1_w"], np.float32).reshape(3, 128).T
    gnc2[:, 0:3] = np.asarray(inputs["gn2_w"], np.float32).reshape(3, 128).T

    shared = dict(
        w1s=np.ascontiguousarray(
            w1.reshape(2, NH, HC).transpose(1, 0, 2)).reshape(NH, 2 * HC)
            .astype(ml_dtypes.bfloat16),
        w2s=np.ascontiguousarray(
            w2.reshape(3, 128, HC).transpose(1, 0, 2)).reshape(128, 3 * HC)
            .astype(ml_dtypes.bfloat16),
        was1=np.ascontiguousarray(
            was1.reshape(2, NH, 12).transpose(1, 0, 2)).reshape(NH, 24)
            .astype(ml_dtypes.bfloat16),
        was2=np.ascontiguousarray(
            was2.reshape(3, 128, 12).transpose(1, 0, 2)).reshape(128, 36)
            .astype(ml_dtypes.bfloat16),
        gncol=gnc1, gncol2=gnc2,
        lin1s=lin1s, head64=head64,
        lin2w=np.asarray(inputs["lin2_w"], np.float32),
        lin2b=np.asarray(inputs["lin2_b"], np.float32).reshape(NCLS, 1),
    )

    in_maps = []
    for core in range(NCORES):
        gs = slice(core * GPC, (core + 1) * GPC)
        m = dict(shared)
        m["xb"] = cm(xg[gs])                           # [114, (fc, g, n)]
        m["mm"] = cm(Mall[gs].transpose(0, 2, 1))      # [114 (s), (sc, g, d)]
        in_maps.append(m)
    return in_maps


_cached_nc = None


def kernel(**inputs):
    global _cached_nc
    in_maps = _host_prep(inputs)
    if _cached_nc is None:
        _cached_nc = _build_program()
    nc = _cached_nc
    res = bass_utils.run_bass_kernel_spmd(nc, in_maps, core_ids=list(range(NCORES)))
    _last_results["exec_time_ns"] = res.exec_time_ns
    _last_results["res"] = res
    out = np.zeros((B, NCLS), np.float32)
    for core in range(NCORES):
        o = res.results[core]["out"]          # [2, 6]
        out[core * GPC:(core + 1) * GPC, :] = o.T
    return out

